# revision 1
# baseline (speedup 1.0000x reference)
"""Trainium2 Bass kernel for nn_EncodingLayer (dense transformer encoder layer).

Reference computation (B=2, S=2048, H=128, NH=8):
    Q/K/V = per-head full-dim projections of x, scores = QK^T/sqrt(H),
    A = softmax(scores), o = A@V, concat heads, y = o@Wo+bo,
    y = LN1(y), f = relu(relu(y@W1+b1)@W2+b2), out = LN2(y+f).

Sharding: data-parallel over query rows. Core c (of 8) owns batch b=c//4 and
query rows q0=(c%4)*512 .. q0+512 of that batch. Each core computes K/V for
its full batch (4x replicated compute, tiny) and the full epilogue for its
512 rows. No collectives; host concatenates the 8 [512,128] slices.

Within a core the attention runs in "transposed score" layout:
    QT/KT = [e, s] via PE, scores^T[t,s] chunks on PE (fp32r, full rate),
    P^T = exp(scores^T) on ACT straight out of PSUM, o^T accumulated on PE
    with V[t,e] chunks as stationary, softmax denominator via ones-vector
    matmul (sum over t = partition dim), division applied to o^T (tiny).
Since |scores| < ~0.4 for this problem scale, softmax without max-subtraction
is numerically exact; bv folds into o^T after division because softmax rows
sum to one.
"""

import math
import numpy as np
from contextlib import ExitStack

import concourse.bass as bass
import concourse.bacc as bacc
import concourse.mybir as mybir
import concourse.tile as tile
from concourse.bass_utils import run_bass_kernel_spmd
from concourse.masks import make_identity

B, S, H, NH = 2, 2048, 128, 8
F = 2 * H                      # FFN hidden dim (256)
NCORES = 8
SQ = (B * S) // NCORES         # 512 query rows per core
TC = S // 128                  # 16 key/value chunks of 128
LN_EPS = 1e-5
FP32 = mybir.dt.float32
FP32R = mybir.dt.float32r
BF16 = mybir.dt.bfloat16
AF = mybir.ActivationFunctionType
ALU = mybir.AluOpType


def _r(ap):
    return ap.bitcast(FP32R)


def _bcast_ap(ap, parts):
    """Partition-broadcast view of a single-partition AP (for DMA)."""
    return bass.AP(tensor=ap.tensor, offset=ap.offset, ap=[[0, parts]] + list(ap.ap)[1:])


def _ln_tile(nc, pool, out_ap, in_ap, eps_tile, g_bc, beta_bc):
    """LayerNorm over the free dim of a [128, H] tile: out = (x-m)/sqrt(v+eps)*g+b."""
    stats = pool.tile([128, nc.vector.BN_STATS_DIM], FP32, tag="ln_stats")
    nc.vector.bn_stats(out=stats[:], in_=in_ap)
    mv = pool.tile([128, nc.vector.BN_AGGR_DIM], FP32, tag="ln_mv")
    nc.vector.bn_aggr(out=mv[:], in_=stats[:])
    std = pool.tile([128, 1], FP32, tag="ln_std")
    nc.scalar.activation(out=std[:], in_=mv[:, 1:2], func=AF.Sqrt, bias=eps_tile[:])
    nc.vector.reciprocal(out=std[:], in_=std[:])
    tmp = pool.tile([128, H], FP32, tag="ln_tmp")
    nc.vector.tensor_scalar(
        out=tmp[:], in0=in_ap, scalar1=mv[:, 0:1], scalar2=std[:],
        op0=ALU.subtract, op1=ALU.mult,
    )
    nc.vector.tensor_mul(out=tmp[:], in0=tmp[:], in1=g_bc[:])
    nc.vector.tensor_add(out=out_ap, in0=tmp[:], in1=beta_bc[:])


def build_module():
    nc = bacc.Bacc(None)

    xb_d = nc.declare_dram_parameter("xb", [S, H], FP32, isOutput=False)
    xq_d = nc.declare_dram_parameter("xq", [SQ, H], FP32, isOutput=False)
    wq_d = nc.declare_dram_parameter("wq", [NH, H, H], FP32R, isOutput=False)
    bq_d = nc.declare_dram_parameter("bq", [NH, H], FP32, isOutput=False)
    wk_d = nc.declare_dram_parameter("wk", [NH, H, H], FP32R, isOutput=False)
    bk_d = nc.declare_dram_parameter("bk", [NH, H], FP32, isOutput=False)
    wv_d = nc.declare_dram_parameter("wv", [NH, H, H], FP32R, isOutput=False)
    bv_d = nc.declare_dram_parameter("bv", [NH, H], FP32, isOutput=False)
    wo_d = nc.declare_dram_parameter("wo", [NH * H, H], FP32R, isOutput=False)
    bo_d = nc.declare_dram_parameter("bo", [H], FP32, isOutput=False)
    w1_d = nc.declare_dram_parameter("w1", [H, F], FP32R, isOutput=False)
    b1_d = nc.declare_dram_parameter("b1", [F], FP32, isOutput=False)
    w2_d = nc.declare_dram_parameter("w2", [F, H], FP32R, isOutput=False)
    b2_d = nc.declare_dram_parameter("b2", [H], FP32, isOutput=False)
    g1_d = nc.declare_dram_parameter("g1", [H], FP32, isOutput=False)
    be1_d = nc.declare_dram_parameter("beta1", [H], FP32, isOutput=False)
    g2_d = nc.declare_dram_parameter("g2", [H], FP32, isOutput=False)
    be2_d = nc.declare_dram_parameter("beta2", [H], FP32, isOutput=False)
    out_d = nc.declare_dram_parameter("out", [SQ, H], FP32, isOutput=True)

    with tile.TileContext(nc) as tc, ExitStack() as ctx:
        singles = ctx.enter_context(tc.tile_pool(name="singles", bufs=1))
        work = ctx.enter_context(tc.tile_pool(name="work", bufs=3))

        # ---- constants / weights to SBUF ----
        ident = singles.tile([128, 128], FP32)
        make_identity(nc, ident[:])
        ones_st = singles.tile([128, 128], FP32)
        nc.vector.memset(ones_st[:], 1.0)
        ones128 = singles.tile([128, 128], FP32)  # all-ones lhsT: partition sums
        nc.vector.tensor_copy(out=_r(ones128[:]), in_=ones_st[:])
        eps_t = singles.tile([128, 1], FP32)
        nc.vector.memset(eps_t[:], LN_EPS)

        # ---- x into SBUF + transposes xT=[d, S], xqT=[d, SQ] ----
        xb_sb = singles.tile([128, TC, H], FP32)  # (s%128, sc, d)
        xb_r = xb_d[:].rearrange("(sc p) d -> p sc d", p=128)
        for q in range(4):
            nc.sync.dma_start(out=xb_sb[:, 4 * q:4 * (q + 1), :], in_=xb_r[:, 4 * q:4 * (q + 1), :])
        xq_sb = singles.tile([128, SQ // 128, H], FP32)
        nc.sync.dma_start(out=xq_sb[:], in_=xq_d[:].rearrange("(sc p) d -> p sc d", p=128))
        xT = singles.tile([H, S], FP32)
        xqT = singles.tile([H, SQ], FP32)

        wq_sb = singles.tile([H, NH, H], FP32)    # (d, h, e)
        nc.sync.dma_start(out=_r(wq_sb[:]), in_=wq_d[:].rearrange("h d e -> d h e"))
        wk_sb = singles.tile([H, NH, H], FP32)
        nc.sync.dma_start(out=_r(wk_sb[:]), in_=wk_d[:].rearrange("h d e -> d h e"))
        wv_sb = singles.tile([H, NH, H], FP32)
        nc.sync.dma_start(out=_r(wv_sb[:]), in_=wv_d[:].rearrange("h d e -> d h e"))
        wo_sb = singles.tile([H, NH, H], FP32)    # (e, h, j)
        nc.sync.dma_start(out=_r(wo_sb[:]), in_=wo_d[:].rearrange("(h e) j -> e h j", h=NH))
        w1_sb = singles.tile([H, F], FP32)        # (d, f)
        nc.sync.dma_start(out=_r(w1_sb[:]), in_=w1_d[:])
        w2_sb = singles.tile([H, 2, H], FP32)     # (f%128, f//128, j)
        nc.sync.dma_start(out=_r(w2_sb[:]), in_=w2_d[:].rearrange("(c f) j -> f c j", c=2))

        bq_sb = singles.tile([H, NH], FP32)       # (e, h)
        nc.sync.dma_start(out=bq_sb[:], in_=bq_d[:].rearrange("h e -> e h"))
        bk_sb = singles.tile([H, NH], FP32)
        nc.sync.dma_start(out=bk_sb[:], in_=bk_d[:].rearrange("h e -> e h"))
        bv_sb = singles.tile([H, NH], FP32)
        nc.sync.dma_start(out=bv_sb[:], in_=bv_d[:].rearrange("h e -> e h"))
        bo_sb = singles.tile([H, 1], FP32)        # per-partition (j)
        nc.sync.dma_start(out=bo_sb[:], in_=bo_d[:].rearrange("(j o) -> j o", o=1))
        b1_sb = singles.tile([H, 2], FP32)        # (f%128, f//128)
        nc.sync.dma_start(out=b1_sb[:], in_=b1_d[:].rearrange("(c f) -> f c", c=2))
        b2_sb = singles.tile([H, 1], FP32)
        nc.sync.dma_start(out=b2_sb[:], in_=b2_d[:].rearrange("(j o) -> j o", o=1))

        g1_bc = singles.tile([128, H], FP32)      # free-dim vectors broadcast over partitions
        nc.sync.dma_start(out=g1_bc[:], in_=_bcast_ap(g1_d[:].rearrange("(o j) -> o j", o=1), 128))
        be1_bc = singles.tile([128, H], FP32)
        nc.sync.dma_start(out=be1_bc[:], in_=_bcast_ap(be1_d[:].rearrange("(o j) -> o j", o=1), 128))
        g2_bc = singles.tile([128, H], FP32)
        nc.sync.dma_start(out=g2_bc[:], in_=_bcast_ap(g2_d[:].rearrange("(o j) -> o j", o=1), 128))
        be2_bc = singles.tile([128, H], FP32)
        nc.sync.dma_start(out=be2_bc[:], in_=_bcast_ap(be2_d[:].rearrange("(o j) -> o j", o=1), 128))


        # PE matmuls (fused LDWEIGHTS) can carry only ONE semaphore wait in
        # codegen. Each dummy transpose below makes PE observe one DMA/engine
        # semaphore so no later matmul needs to wait on two at once; _zd()
        # writes a [1,1] dummy into a new PSUM pool's first tile so the
        # pool-transition (released-zone) dependency is absorbed there
        # instead of landing on a real matmul that also has a data wait.
        def _zd(tile_ap):
            nc.tensor.matmul(tile_ap[0:1, 0:1], ident[:, 0:1], ident[:, 0:1],
                             start=True, stop=True)

        with tc.tile_pool(name="abs_ps", bufs=7, space="PSUM") as abs_ps:
            for absorber in (
                ident[:], xb_sb[:, 0, :], wv_sb[:, 0, :].bitcast(FP32),
                wq_sb[:, 0, :].bitcast(FP32), wo_sb[:, 0, :].bitcast(FP32),
                w1_sb[:, 0:128].bitcast(FP32), w2_sb[:, 0, :].bitcast(FP32),
            ):
                pt = abs_ps.tile([128, 128], FP32, tag="abs")
                nc.tensor.transpose(pt[:], absorber, ident[:])

        with tc.tile_pool(name="tp_ps", bufs=2, space="PSUM") as tp_ps:
            for sc in range(SQ // 128):
                pt = tp_ps.tile([128, 128], FP32, tag="tp")
                if sc == 0:
                    _zd(pt)
                nc.tensor.transpose(pt[:], xq_sb[:, sc, :], ident[:])
                nc.vector.tensor_copy(out=_r(xqT[:, sc * 128:(sc + 1) * 128]), in_=pt[:])
            for sc in range(TC):
                pt = tp_ps.tile([128, 128], FP32, tag="tp")
                nc.tensor.transpose(pt[:], xb_sb[:, sc, :], ident[:])
                nc.vector.tensor_copy(out=_r(xT[:, sc * 128:(sc + 1) * 128]), in_=pt[:])

        # ---- V for all heads: v_sb[t%128, tc, h, e] = (x @ Wv)[t, (h e)] ----
        v_sb = singles.tile([128, TC, NH, H], FP32)
        with tc.tile_pool(name="v_ps", bufs=2, space="PSUM") as v_ps:
            for tcc in range(TC):
                vp = v_ps.tile([128, NH * H], FP32, tag="v")
                if tcc == 0:
                    _zd(vp)
                for half in range(2):
                    nc.tensor.matmul(
                        vp[:, half * 512:(half + 1) * 512],
                        _r(xT[:, tcc * 128:(tcc + 1) * 128]),
                        _r(wv_sb[:, half * 4:(half + 1) * 4, :]),
                        start=True, stop=True,
                    )
                nc.vector.tensor_copy(out=_r(v_sb[:, tcc, :, :]), in_=vp[:])

        # ---- attention head loop ----
        kt_pool = ctx.enter_context(tc.tile_pool(name="kt", bufs=2))
        qt_pool = ctx.enter_context(tc.tile_pool(name="qt", bufs=2))
        pt_pool = ctx.enter_context(tc.tile_pool(name="pt", bufs=3))
        ot_pool = ctx.enter_context(tc.tile_pool(name="ot", bufs=2))

        yT_sb = singles.tile([H, SQ], FP32)  # attention block output (pre-LN), [j, s]

        with (
            tc.tile_pool(name="s_ps", bufs=2, space="PSUM") as s_ps,
            tc.tile_pool(name="o_ps", bufs=2, space="PSUM") as o_ps,
            tc.tile_pool(name="d_ps", bufs=1, space="PSUM") as d_ps,
            tc.tile_pool(name="y_ps", bufs=1, space="PSUM") as y_ps,
        ):
            y_acc = y_ps.tile([H, SQ], FP32)
            _zd(y_acc)

            # o^T = o_acc / denom + bv (softmax rows sum to 1). Emitted at the
            # START of the next head so the DVE chain runs while PE streams the
            # next head's matmuls, and the Wo matmul is emitted AFTER that
            # head's t-loop so in-order PE never stalls on it.
            def _finalize_dve(hp, o_p, d_p):
                rec_bc = ot_pool.tile([128, SQ], FP32, tag="rec")
                scr = ot_pool.tile([128, SQ], FP32, tag="rec_scr")
                nc.vector.reciprocal_approx_accurate(out=rec_bc[:], in_=d_p[:], scratch=scr[:])
                oT = ot_pool.tile([H, SQ], FP32, tag="oT")
                nc.vector.tensor_mul(out=_r(oT[:]), in0=o_p[:], in1=rec_bc[:])
                nc.vector.tensor_scalar_add(out=_r(oT[:]), in0=oT[:],
                                            scalar1=bv_sb[:, hp:hp + 1])
                return oT

            prev = None  # (h, o_acc, d_acc)
            for h in range(NH):
                oT_prev = None

                # K^T[e, t] and Q^T[e, s] with biases (1/sqrt(H) folded into Q)
                kt = kt_pool.tile([H, S], BF16, tag="kt")
                for i in range(S // 512):
                    kp = s_ps.tile([128, 1024], FP32, tag="s")
                    if h == 0 and i == 0:
                        _zd(kp)
                    nc.tensor.matmul(
                        kp[:, 0:512], _r(wk_sb[:, h, :]), _r(xT[:, i * 512:(i + 1) * 512]),
                        start=True, stop=True,
                    )
                    nc.vector.tensor_scalar_add(
                        out=kt[:, i * 512:(i + 1) * 512], in0=kp[:, 0:512],
                        scalar1=bk_sb[:, h:h + 1],
                    )
                qt = qt_pool.tile([H, SQ], BF16, tag="qt")
                qp = s_ps.tile([128, 1024], FP32, tag="s")
                nc.tensor.matmul(qp[:, 0:512], _r(wq_sb[:, h, :]), _r(xqT[:]),
                                 start=True, stop=True)
                nc.vector.tensor_scalar(
                    out=qt[:], in0=qp[:, 0:512], scalar1=bq_sb[:, h:h + 1],
                    scalar2=1.0 / math.sqrt(H), op0=ALU.add, op1=ALU.mult,
                )

                o_acc = o_ps.tile([H, SQ], FP32, tag="o")
                d_acc = d_ps.tile([128, SQ], FP32, tag="d")
                if h == 0:
                    _zd(o_acc)
                    _zd(d_acc)

                # Software-pipelined t-loop: PE is in-order, so emit the
                # NEXT group's scores before this group's denominator/PV
                # matmuls — PE then has ~1.4us of work queued while ACT
                # computes exp(g), instead of stalling behind it.
                def _scores(g):
                    sp = s_ps.tile([128, 1024], FP32, tag="s")
                    for j in range(2):
                        tcc = 2 * g + j
                        nc.tensor.matmul(
                            sp[:, j * 512:(j + 1) * 512],
                            kt[:, tcc * 128:(tcc + 1) * 128], qt[:],
                            start=True, stop=True,
                        )
                    pt = pt_pool.tile([128, 1024], FP32, tag="pt")
                    nc.scalar.activation(out=_r(pt[:]), in_=sp[:], func=AF.Exp)
                    return pt

                def _denom_pv(g, pt):
                    for j in range(2):
                        tcc = 2 * g + j
                        nc.tensor.matmul(
                            d_acc[:], _r(ones128[:]), _r(pt[:, j * 512:(j + 1) * 512]),
                            start=(tcc == 0), stop=(tcc == TC - 1),
                        )
                        nc.tensor.matmul(
                            o_acc[:], _r(v_sb[:, tcc, h, :]), _r(pt[:, j * 512:(j + 1) * 512]),
                            start=(tcc == 0), stop=(tcc == TC - 1),
                        )

                pt_cur = _scores(0)
                for g in range(TC // 2):
                    if g == 2 and prev is not None:
                        oT_prev = _finalize_dve(*prev)
                    pt_next = _scores(g + 1) if g + 1 < TC // 2 else None
                    _denom_pv(g, pt_cur)
                    pt_cur = pt_next

                if prev is not None:
                    nc.tensor.matmul(y_acc[:], _r(wo_sb[:, prev[0], :]), _r(oT_prev[:]),
                                     start=(prev[0] == 0), stop=False)
                prev = (h, o_acc, d_acc)

            oT_last = _finalize_dve(*prev)
            nc.tensor.matmul(y_acc[:], _r(wo_sb[:, NH - 1, :]), _r(oT_last[:]),
                             start=False, stop=True)
            nc.vector.tensor_scalar_add(out=yT_sb[:], in0=y_acc[:], scalar1=bo_sb[:])

        # ---- epilogue: transpose y, LN1, FFN (transposed), residual, LN2 ----
        y1_sb = singles.tile([128, SQ // 128, H], FP32)   # LN1 output, natural (s, j)
        y1T = singles.tile([H, SQ], FP32)                 # LN1 output, [d, s]
        out_sb = singles.tile([128, SQ // 128, H], FP32)

        with (
            tc.tile_pool(name="e_ps", bufs=2, space="PSUM") as e_ps,
            tc.tile_pool(name="u_ps", bufs=2, space="PSUM") as u_ps,
            tc.tile_pool(name="z_ps", bufs=1, space="PSUM") as z_ps,
        ):
            for sc in range(SQ // 128):
                yp = e_ps.tile([128, 128], FP32, tag="e")
                if sc == 0:
                    _zd(yp)
                nc.tensor.transpose(yp[:], yT_sb[:, sc * 128:(sc + 1) * 128], ident[:])
                _ln_tile(nc, work, y1_sb[:, sc, :], yp[:], eps_t, g1_bc, be1_bc)
            for sc in range(SQ // 128):
                yp = e_ps.tile([128, 128], FP32, tag="e")
                nc.tensor.transpose(yp[:], y1_sb[:, sc, :], ident[:])
                nc.vector.tensor_copy(out=_r(y1T[:, sc * 128:(sc + 1) * 128]), in_=yp[:])

            # u^T[f, s] = relu(W1^T y1 + b1), f in two 128-chunks
            uT = work.tile([H, 2, SQ], FP32, tag="uT")
            for fc in range(2):
                up = u_ps.tile([128, SQ], FP32, tag="u")
                if fc == 0:
                    _zd(up)
                nc.tensor.matmul(up[:], _r(w1_sb[:, fc * 128:(fc + 1) * 128]), _r(y1T[:]),
                                 start=True, stop=True)
                nc.scalar.activation(out=_r(uT[:, fc, :]), in_=up[:], func=AF.Relu,
                                     bias=b1_sb[:, fc:fc + 1])
            # z^T[j, s] = relu(W2^T u + b2)
            zp = z_ps.tile([H, SQ], FP32, tag="z")
            _zd(zp)
            for fc in range(2):
                nc.tensor.matmul(zp[:], _r(w2_sb[:, fc, :]), _r(uT[:, fc, :]),
                                 start=(fc == 0), stop=(fc == 1))
            zT = work.tile([H, SQ], FP32, tag="zT")
            nc.scalar.activation(out=zT[:], in_=zp[:], func=AF.Relu, bias=b2_sb[:])

            # residual + LN2, back in natural layout
            for sc in range(SQ // 128):
                rp = e_ps.tile([128, 128], FP32, tag="e")
                nc.tensor.transpose(rp[:], zT[:, sc * 128:(sc + 1) * 128], ident[:])
                r_sb = work.tile([128, H], FP32, tag="r_sb")
                nc.vector.tensor_add(out=r_sb[:], in0=rp[:], in1=y1_sb[:, sc, :])
                _ln_tile(nc, work, out_sb[:, sc, :], r_sb[:], eps_t, g2_bc, be2_bc)

        nc.sync.dma_start(out=out_d[:].rearrange("(sc p) j -> p sc j", p=128), in_=out_sb[:])

    nc.finalize()
    return nc


_CACHE: dict = {}


def _get_nc():
    if "nc" not in _CACHE:
        _CACHE["nc"] = build_module()
    return _CACHE["nc"]


def _in_maps(inputs):
    f32 = lambda a: np.ascontiguousarray(np.asarray(a), dtype=np.float32)
    x = f32(inputs["x"])
    shared = {
        "wq": f32(inputs["Wq"]), "bq": f32(inputs["bq"]),
        "wk": f32(inputs["Wk"]), "bk": f32(inputs["bk"]),
        "wv": f32(inputs["Wv"]), "bv": f32(inputs["bv"]),
        "wo": f32(inputs["Wo"]), "bo": f32(inputs["bo"]),
        "w1": f32(inputs["W1"]), "b1": f32(inputs["b1"]),
        "w2": f32(inputs["W2"]), "b2": f32(inputs["b2"]),
        "g1": f32(inputs["g1"]), "beta1": f32(inputs["beta1"]),
        "g2": f32(inputs["g2"]), "beta2": f32(inputs["beta2"]),
    }
    maps = []
    for c in range(NCORES):
        b, qi = divmod(c, NCORES // B)
        q0 = qi * SQ
        maps.append({
            "xb": np.ascontiguousarray(x[b]),
            "xq": np.ascontiguousarray(x[b, q0:q0 + SQ]),
            **shared,
        })
    return maps


def run(inputs, **kwargs):
    nc = _get_nc()
    res = run_bass_kernel_spmd(nc, _in_maps(inputs), core_ids=list(range(NCORES)), **kwargs)
    parts = [res.results[c]["out"] for c in range(NCORES)]
    y = np.concatenate(parts, axis=0).reshape(B, S, H).astype(np.float32)
    return y, res


def kernel(**inputs) -> np.ndarray:
    y, _ = run(inputs)
    return y



# revision 15
# speedup vs baseline: 1.1386x; 1.1386x over previous
"""Trainium2 Bass kernel for nn_EncodingLayer (dense transformer encoder layer).

Reference computation (B=2, S=2048, H=128, NH=8):
    Q/K/V = per-head full-dim projections of x, scores = QK^T/sqrt(H),
    A = softmax(scores), o = A@V, concat heads, y = o@Wo+bo,
    y = LN1(y), f = relu(relu(y@W1+b1)@W2+b2), out = LN2(y+f).

Sharding: data-parallel over query rows. Core c (of 8) owns batch b=c//4 and
query rows q0=(c%4)*512 .. q0+512 of that batch. Each core computes K/V for
its full batch (4x replicated compute, tiny) and the full epilogue for its
512 rows. No collectives; host concatenates the 8 [512,128] slices.

Within a core:
  Phase 1 (projections, dense PE stream): x transposes, then V for all heads
    (natural [t, h, e], bf16), then K^T/Q^T per head (bf16), with PSUM drains
    split across ACT/DVE/GpSimd so PE never throttles on a single drainer.
  Phase 2 (attention): per head a pure t-loop: scores^T chunks on PE (bf16),
    P^T = exp on ACT straight out of PSUM, denominator via ones-vector matmul
    (sum over the t partition dim), o^T accumulated on PE. Softmax without
    max-subtraction is numerically exact at this problem scale (|scores|<~.4),
    and bv folds into o^T after division since softmax rows sum to one.
  Phase 3 (epilogue, fully transposed): LN1/FFN/LN2 all in [feature, seq]
    layout. LN stats over the feature dim = partition dim via ones-matmul
    column sums, so no transposes are needed until the final output.
"""

import math
import numpy as np
from contextlib import ExitStack

import concourse.bass as bass
import concourse.bacc as bacc
import concourse.mybir as mybir
import concourse.tile as tile
from concourse.bass_utils import run_bass_kernel_spmd
from concourse.masks import make_identity

B, S, H, NH = 2, 2048, 128, 8
F = 2 * H                      # FFN hidden dim (256)
NCORES = 8
SQ = (B * S) // NCORES         # 512 query rows per core
TC = S // 128                  # 16 key/value chunks of 128
LN_EPS = 1e-5
FP32 = mybir.dt.float32
FP32R = mybir.dt.float32r
BF16 = mybir.dt.bfloat16
AF = mybir.ActivationFunctionType
ALU = mybir.AluOpType


def _r(ap):
    return ap.bitcast(FP32R)


def build_module():
    nc = bacc.Bacc(None)

    xb_d = nc.declare_dram_parameter("xb", [S, H], FP32, isOutput=False)
    xq_d = nc.declare_dram_parameter("xq", [SQ, H], FP32, isOutput=False)
    wq_d = nc.declare_dram_parameter("wq", [NH, H, H], FP32R, isOutput=False)
    bq_d = nc.declare_dram_parameter("bq", [NH, H], FP32, isOutput=False)
    wk_d = nc.declare_dram_parameter("wk", [NH, H, H], FP32R, isOutput=False)
    bk_d = nc.declare_dram_parameter("bk", [NH, H], FP32, isOutput=False)
    wv_d = nc.declare_dram_parameter("wv", [NH, H, H], FP32R, isOutput=False)
    bv_d = nc.declare_dram_parameter("bv", [NH, H], FP32, isOutput=False)
    wo_d = nc.declare_dram_parameter("wo", [NH * H, H], FP32R, isOutput=False)
    bo_d = nc.declare_dram_parameter("bo", [H], FP32, isOutput=False)
    w1_d = nc.declare_dram_parameter("w1", [H, F], FP32R, isOutput=False)
    b1_d = nc.declare_dram_parameter("b1", [F], FP32, isOutput=False)
    w2_d = nc.declare_dram_parameter("w2", [F, H], FP32R, isOutput=False)
    b2_d = nc.declare_dram_parameter("b2", [H], FP32, isOutput=False)
    g1_d = nc.declare_dram_parameter("g1", [H], FP32, isOutput=False)
    be1_d = nc.declare_dram_parameter("beta1", [H], FP32, isOutput=False)
    g2_d = nc.declare_dram_parameter("g2", [H], FP32, isOutput=False)
    be2_d = nc.declare_dram_parameter("beta2", [H], FP32, isOutput=False)
    out_d = nc.declare_dram_parameter("out", [SQ, H], FP32, isOutput=True)

    with tile.TileContext(nc) as tc, ExitStack() as ctx:
        singles = ctx.enter_context(tc.tile_pool(name="singles", bufs=1))
        work = ctx.enter_context(tc.tile_pool(name="work", bufs=3))

        # ---- constants ----
        ident = singles.tile([128, 128], FP32)
        make_identity(nc, ident[:])
        ones128 = singles.tile([128, 128], FP32)  # all-ones lhsT: partition sums
        nc.vector.memset(ones128[:], 1.0)
        ones_bf = singles.tile([128, 128], BF16)  # bf16 twin for bf16 matmuls
        nc.vector.memset(ones_bf[:], 1.0)
        eps_t = singles.tile([128, 1], FP32)
        nc.vector.memset(eps_t[:], LN_EPS)

        # ---- x into SBUF (chunked DMAs so transposes can start early) ----
        xb_sb = singles.tile([128, TC, H], FP32)  # (s%128, sc, d)
        xb_r = xb_d[:].rearrange("(sc p) d -> p sc d", p=128)
        for q in range(4):
            nc.sync.dma_start(out=xb_sb[:, 4 * q:4 * (q + 1), :], in_=xb_r[:, 4 * q:4 * (q + 1), :])
        xq_sb = singles.tile([128, SQ // 128, H], FP32)
        nc.sync.dma_start(out=xq_sb[:], in_=xq_d[:].rearrange("(sc p) d -> p sc d", p=128))

        # ---- weights (wv first: V matmuls are emitted first) ----
        wv_sb = singles.tile([H, NH, H], FP32)    # (d, h, e)
        for half in range(2):
            nc.sync.dma_start(out=_r(wv_sb[:, 4 * half:4 * (half + 1), :]),
                              in_=wv_d[:].rearrange("h d e -> d h e")[:, 4 * half:4 * (half + 1), :])
        wk_sb = singles.tile([H, NH, H], FP32)
        nc.sync.dma_start(out=_r(wk_sb[:]), in_=wk_d[:].rearrange("h d e -> d h e"))
        wq_sb = singles.tile([H, NH, H], FP32)
        nc.sync.dma_start(out=_r(wq_sb[:]), in_=wq_d[:].rearrange("h d e -> d h e"))
        wo_sb = singles.tile([H, NH, H], FP32)    # (e, h, j)
        nc.sync.dma_start(out=_r(wo_sb[:]), in_=wo_d[:].rearrange("(h e) j -> e h j", h=NH))
        w1_sb = singles.tile([H, F], FP32)        # (d, f)
        nc.sync.dma_start(out=_r(w1_sb[:]), in_=w1_d[:])
        w2_sb = singles.tile([H, 2, H], FP32)     # (f%128, f//128, j)
        nc.sync.dma_start(out=_r(w2_sb[:]), in_=w2_d[:].rearrange("(c f) j -> f c j", c=2))

        bq_sb = singles.tile([H, NH], FP32)       # (e, h)
        nc.sync.dma_start(out=bq_sb[:], in_=bq_d[:].rearrange("h e -> e h"))
        bk_sb = singles.tile([H, NH], FP32)
        nc.sync.dma_start(out=bk_sb[:], in_=bk_d[:].rearrange("h e -> e h"))
        bv_sb = singles.tile([H, NH], FP32)
        nc.sync.dma_start(out=bv_sb[:], in_=bv_d[:].rearrange("h e -> e h"))
        bo_sb = singles.tile([H, 1], FP32)        # per-partition (j)
        nc.sync.dma_start(out=bo_sb[:], in_=bo_d[:].rearrange("(j o) -> j o", o=1))
        b1_sb = singles.tile([H, 2], FP32)        # (f%128, f//128)
        nc.sync.dma_start(out=b1_sb[:], in_=b1_d[:].rearrange("(c f) -> f c", c=2))
        b2_sb = singles.tile([H, 1], FP32)
        nc.sync.dma_start(out=b2_sb[:], in_=b2_d[:].rearrange("(j o) -> j o", o=1))
        g1_sb = singles.tile([H, 1], FP32)        # per-partition (j) in transposed LN
        nc.sync.dma_start(out=g1_sb[:], in_=g1_d[:].rearrange("(j o) -> j o", o=1))
        be1_sb = singles.tile([H, 1], FP32)
        nc.sync.dma_start(out=be1_sb[:], in_=be1_d[:].rearrange("(j o) -> j o", o=1))
        g2_sb = singles.tile([H, 1], FP32)
        nc.sync.dma_start(out=g2_sb[:], in_=g2_d[:].rearrange("(j o) -> j o", o=1))
        be2_sb = singles.tile([H, 1], FP32)
        nc.sync.dma_start(out=be2_sb[:], in_=be2_d[:].rearrange("(j o) -> j o", o=1))

        xT = singles.tile([H, S], FP32)
        xqT = singles.tile([H, SQ], FP32)

        # PE matmuls (fused LDWEIGHTS) can carry only ONE semaphore wait in
        # codegen. Each dummy transpose below makes PE observe one DMA/engine
        # semaphore so no later matmul needs to wait on two at once; _zd()
        # writes a [1,1] dummy into a new PSUM pool's first tile so the
        # pool-transition (released-zone) dependency is absorbed there
        # instead of landing on a real matmul that also has a data wait.
        def _zd(tile_ap):
            nc.tensor.matmul(tile_ap[0:1, 0:1], ident[:, 0:1], ident[:, 0:1],
                             start=True, stop=True)

        with tc.tile_pool(name="abs_ps", bufs=8, space="PSUM") as abs_ps:
            for absorber in (
                ident[:], xb_sb[:, 0, :], wv_sb[:, 0, :].bitcast(FP32),
                wk_sb[:, 0, :].bitcast(FP32), wq_sb[:, 0, :].bitcast(FP32),
                wo_sb[:, 0, :].bitcast(FP32), w1_sb[:, 0:128].bitcast(FP32),
                w2_sb[:, 0, :].bitcast(FP32),
            ):
                pt = abs_ps.tile([128, 128], FP32, tag="abs")
                nc.tensor.transpose(pt[:], absorber, ident[:])

        # ---- transposes: xT=[d, S], xqT=[d, SQ] ----
        with tc.tile_pool(name="tp_ps", bufs=2, space="PSUM") as tp_ps:
            for sc in range(SQ // 128):
                pt = tp_ps.tile([128, 128], FP32, tag="tp")
                if sc == 0:
                    _zd(pt)
                nc.tensor.transpose(pt[:], xq_sb[:, sc, :], ident[:])
                nc.vector.tensor_copy(out=_r(xqT[:, sc * 128:(sc + 1) * 128]), in_=pt[:])
            for sc in range(TC):
                pt = tp_ps.tile([128, 128], FP32, tag="tp")
                nc.tensor.transpose(pt[:], xb_sb[:, sc, :], ident[:])
                nc.vector.tensor_copy(out=_r(xT[:, sc * 128:(sc + 1) * 128]), in_=pt[:])

        # ---- projection phase: V (all heads), then K^T/Q^T per head ----
        v_sb = singles.tile([128, TC, NH, H], BF16)   # (t%128, tc, h, e)
        kt_all = singles.tile([H, NH, S], BF16)       # (e, h, t)
        qt_all = singles.tile([H, NH, SQ], BF16)      # (e, h, s)

        with (
            tc.tile_pool(name="v_ps", bufs=2, space="PSUM") as v_ps,
            tc.tile_pool(name="kq_ps", bufs=2, space="PSUM") as kq_ps,
        ):
            # V natural: vp[t, (h e)] = x @ Wv. Drains split ACT/DVE/GpSimd.
            for tcc in range(TC):
                vp = v_ps.tile([128, NH * H], FP32, tag="v")
                if tcc == 0:
                    _zd(vp)
                for half in range(2):
                    nc.tensor.matmul(
                        vp[:, half * 512:(half + 1) * 512],
                        _r(xT[:, tcc * 128:(tcc + 1) * 128]),
                        _r(wv_sb[:, half * 4:(half + 1) * 4, :]),
                        start=True, stop=True,
                    )
                dst = v_sb[:, tcc, :, :]
                if tcc < 11:
                    nc.scalar.activation(out=dst, in_=vp[:], func=AF.Copy)
                else:
                    nc.vector.tensor_copy(out=dst, in_=vp[:])

            # K^T[e, t] per head (2 tiles of 1024 t-cols), Q^T[e, s] packed
            # two heads per PSUM tile. All K/Q drains on DVE so attention
            # matmuls observe a single (DVE) semaphore for kt+qt.
            for h in range(NH):
                for i in range(2):
                    kp = kq_ps.tile([128, 1024], FP32, tag="kq")
                    if h == 0 and i == 0:
                        _zd(kp)
                    for j in range(2):
                        nc.tensor.matmul(
                            kp[:, j * 512:(j + 1) * 512],
                            _r(wk_sb[:, h, :]),
                            _r(xT[:, (2 * i + j) * 512:(2 * i + j + 1) * 512]),
                            start=True, stop=True,
                        )
                    nc.vector.tensor_scalar_add(
                        out=kt_all[:, h, i * 1024:(i + 1) * 1024], in0=kp[:],
                        scalar1=bk_sb[:, h:h + 1],
                    )
                if h % 2 == 1:
                    qp = kq_ps.tile([128, 1024], FP32, tag="kq")
                    for hh in (h - 1, h):
                        nc.tensor.matmul(
                            qp[:, (hh % 2) * 512:(hh % 2 + 1) * 512],
                            _r(wq_sb[:, hh, :]), _r(xqT[:]),
                            start=True, stop=True,
                        )
                    for hh in (h - 1, h):
                        nc.vector.tensor_scalar(
                            out=qt_all[:, hh, :], in0=qp[:, (hh % 2) * 512:(hh % 2 + 1) * 512],
                            scalar1=bq_sb[:, hh:hh + 1], scalar2=1.0 / math.sqrt(H),
                            op0=ALU.add, op1=ALU.mult,
                        )

        # ---- attention head loop (pure t-loops) ----
        pt_pool = ctx.enter_context(tc.tile_pool(name="pt", bufs=3))
        ot_pool = ctx.enter_context(tc.tile_pool(name="ot", bufs=2))

        yT_sb = singles.tile([H, SQ], FP32)  # attention block output (pre-LN), [j, s]

        with (
            tc.tile_pool(name="s_ps", bufs=2, space="PSUM") as s_ps,
            tc.tile_pool(name="o_ps", bufs=2, space="PSUM") as o_ps,
            tc.tile_pool(name="d_ps", bufs=1, space="PSUM") as d_ps,
            tc.tile_pool(name="y_ps", bufs=1, space="PSUM") as y_ps,
        ):
            y_acc = y_ps.tile([H, SQ], FP32)
            _zd(y_acc)
            # Absorb the DVE watermark for the last projection drain (qt of
            # head 7 — DVE is in-order, so this covers every V/K/Q drain)
            # once, before the head loops, so attention matmuls carry at most
            # one un-observed wait (ACT, for the pt tiles).
            gp_abs = s_ps.tile([128, 1024], FP32, tag="s")
            _zd(gp_abs)
            q_view = qt_all[0:1, NH - 1, 0:2].bitcast(FP32)[:, 0:1]
            nc.tensor.matmul(gp_abs[0:1, 0:1], q_view, q_view, start=True, stop=True)

            # o^T = o_acc / denom + bv (softmax rows sum to 1). Emitted at the
            # START of the next head so the DVE chain runs while PE streams the
            # next head's matmuls, and the Wo matmul is emitted AFTER that
            # head's t-loop so in-order PE never stalls on it.
            def _finalize_dve(hp, o_p, d_p):
                rec_bc = ot_pool.tile([128, SQ], FP32, tag="rec")
                scr = ot_pool.tile([128, SQ], FP32, tag="rec_scr")
                nc.vector.reciprocal_approx_accurate(out=rec_bc[:], in_=d_p[:], scratch=scr[:])
                oT = ot_pool.tile([H, SQ], FP32, tag="oT")
                nc.vector.tensor_mul(out=_r(oT[:]), in0=o_p[:], in1=rec_bc[:])
                nc.vector.tensor_scalar_add(out=_r(oT[:]), in0=oT[:],
                                            scalar1=bv_sb[:, hp:hp + 1])
                return oT

            prev = None  # (h, o_acc, d_acc)
            for h in range(NH):
                oT_prev = None
                kt = kt_all[:, h, :]
                qt = qt_all[:, h, :]

                o_acc = o_ps.tile([H, SQ], FP32, tag="o")
                d_acc = d_ps.tile([128, SQ], FP32, tag="d")
                if h == 0:
                    _zd(o_acc)
                    _zd(d_acc)

                # Software-pipelined t-loop: PE is in-order, so emit the
                # NEXT group's scores before this group's denominator/PV
                # matmuls — PE then has ~1.4us of work queued while ACT
                # computes exp(g), instead of stalling behind it.
                def _scores(g):
                    sp = s_ps.tile([128, 1024], FP32, tag="s")
                    for j in range(2):
                        tcc = 2 * g + j
                        nc.tensor.matmul(
                            sp[:, j * 512:(j + 1) * 512],
                            kt[:, tcc * 128:(tcc + 1) * 128], qt,
                            start=True, stop=True,
                        )
                    pt = pt_pool.tile([128, 1024], BF16, tag="pt")
                    nc.scalar.activation(out=pt[:], in_=sp[:], func=AF.Exp)
                    return pt

                def _denom_pv(g, pt):
                    for j in range(2):
                        tcc = 2 * g + j
                        nc.tensor.matmul(
                            d_acc[:], ones_bf[:], pt[:, j * 512:(j + 1) * 512],
                            start=(tcc == 0), stop=(tcc == TC - 1),
                        )
                        nc.tensor.matmul(
                            o_acc[:], v_sb[:, tcc, h, :], pt[:, j * 512:(j + 1) * 512],
                            start=(tcc == 0), stop=(tcc == TC - 1),
                        )

                pt_cur = _scores(0)
                for g in range(TC // 2):
                    if g == 2 and prev is not None:
                        oT_prev = _finalize_dve(*prev)
                    pt_next = _scores(g + 1) if g + 1 < TC // 2 else None
                    _denom_pv(g, pt_cur)
                    pt_cur = pt_next

                if prev is not None:
                    nc.tensor.matmul(y_acc[:], _r(wo_sb[:, prev[0], :]), _r(oT_prev[:]),
                                     start=(prev[0] == 0), stop=False)
                prev = (h, o_acc, d_acc)

            oT_last = _finalize_dve(*prev)
            nc.tensor.matmul(y_acc[:], _r(wo_sb[:, NH - 1, :]), _r(oT_last[:]),
                             start=False, stop=True)
            nc.vector.tensor_scalar_add(out=_r(yT_sb[:]), in0=y_acc[:], scalar1=bo_sb[:])

        # ---- epilogue, fully transposed: LN over the feature dim (= the
        # partition dim here) via ones-matmul column sums. For a [j, s] tile:
        #   sum_y = 1^T yT, sum_q = 1^T (yT*yT)   (PE, broadcast to all rows)
        #   m = sum_y/128, var = (sum_q - m*sum_y)/128
        #   out = (yT - m) * rsqrt(var+eps) * g[j] + beta[j]
        epi = ctx.enter_context(tc.tile_pool(name="epi", bufs=1))

        def _ln_T(out_ap, in_sb, sq_ps_pool, g_col, beta_col):
            ysq = epi.tile([H, SQ], FP32, tag="sq")
            nc.scalar.activation(out=_r(ysq[:]), in_=in_sb, func=AF.Square)
            sum_y = sq_ps_pool.tile([128, SQ], FP32, tag="sy")
            nc.tensor.matmul(sum_y[:], _r(ones128[:]), _r(in_sb), start=True, stop=True)
            sum_q = sq_ps_pool.tile([128, SQ], FP32, tag="sq")
            nc.tensor.matmul(sum_q[:], _r(ones128[:]), _r(ysq[:]), start=True, stop=True)
            # m = sum_y/128 (to SBUF: DVE reads at most one PSUM input/op),
            # t1 = m^2, var = sum_q/128 - m^2, std = sqrt(var + eps)
            m_sb = epi.tile([128, SQ], FP32, tag="m")
            nc.vector.tensor_scalar_mul(out=m_sb[:], in0=sum_y[:], scalar1=1.0 / H)
            t1 = epi.tile([128, SQ], FP32, tag="t1")
            nc.vector.scalar_tensor_tensor(
                out=t1[:], in0=sum_y[:], scalar=1.0 / H, in1=m_sb[:],
                op0=ALU.mult, op1=ALU.mult,
            )
            nc.vector.scalar_tensor_tensor(
                out=t1[:], in0=sum_q[:], scalar=1.0 / H, in1=t1[:],
                op0=ALU.mult, op1=ALU.subtract,
            )
            std = epi.tile([128, SQ], FP32, tag="std")
            nc.scalar.activation(out=std[:], in_=t1[:], func=AF.Sqrt,
                                 bias=eps_t[:])
            rstd = epi.tile([128, SQ], FP32, tag="rs")
            nc.vector.reciprocal(out=rstd[:], in_=std[:])
            # ctr = yT - m
            ctr = epi.tile([128, SQ], FP32, tag="ctr")
            nc.vector.tensor_sub(out=ctr[:], in0=in_sb, in1=m_sb[:])
            nc.vector.tensor_mul(out=ctr[:], in0=ctr[:], in1=rstd[:])
            nc.vector.tensor_scalar(
                out=out_ap, in0=ctr[:], scalar1=g_col[:], scalar2=beta_col[:],
                op0=ALU.mult, op1=ALU.add,
            )

        y1T = singles.tile([H, SQ], FP32)   # LN1 output, [j, s]
        uT = singles.tile([H, 2, SQ], FP32)
        rT = singles.tile([H, SQ], FP32)
        outT = singles.tile([H, SQ], FP32)
        out_sb = singles.tile([128, SQ // 128, H], FP32)

        with (
            tc.tile_pool(name="st_ps", bufs=1, space="PSUM") as st_ps,
            tc.tile_pool(name="u_ps", bufs=2, space="PSUM") as u_ps,
            tc.tile_pool(name="e_ps", bufs=2, space="PSUM") as e_ps,
        ):
            dummy = st_ps.tile([128, 1], FP32, tag="zd")
            _zd(dummy)
            _ln_T(_r(y1T[:]), yT_sb[:], st_ps, g1_sb, be1_sb)

            # u^T[f, s] = relu(W1^T y1 + b1), f in two 128-chunks
            for fc in range(2):
                up = u_ps.tile([128, SQ], FP32, tag="u")
                if fc == 0:
                    _zd(up)
                nc.tensor.matmul(up[:], _r(w1_sb[:, fc * 128:(fc + 1) * 128]), _r(y1T[:]),
                                 start=True, stop=True)
                nc.scalar.activation(out=_r(uT[:, fc, :]), in_=up[:], func=AF.Relu,
                                     bias=b1_sb[:, fc:fc + 1])
            # z^T[j, s] = relu(W2^T u + b2)
            zp = u_ps.tile([H, SQ], FP32, tag="u")
            for fc in range(2):
                nc.tensor.matmul(zp[:], _r(w2_sb[:, fc, :]), _r(uT[:, fc, :]),
                                 start=(fc == 0), stop=(fc == 1))
            # residual in transposed space: rT = y1T + relu(zp + b2)
            nc.scalar.activation(out=_r(rT[:]), in_=zp[:], func=AF.Relu, bias=b2_sb[:])
            nc.vector.tensor_add(out=_r(rT[:]), in0=rT[:], in1=y1T[:])

            _ln_T(outT[:], rT[:], st_ps, g2_sb, be2_sb)

            # back to natural layout + store
            for sc in range(SQ // 128):
                op = e_ps.tile([128, 128], FP32, tag="e")
                if sc == 0:
                    _zd(op)
                nc.tensor.transpose(op[:], outT[:, sc * 128:(sc + 1) * 128], ident[:])
                nc.vector.tensor_copy(out=out_sb[:, sc, :], in_=op[:])
                nc.sync.dma_start(
                    out=out_d[:].rearrange("(sc p) j -> p sc j", p=128)[:, sc:sc + 1, :],
                    in_=out_sb[:, sc:sc + 1, :],
                )

    nc.finalize()
    return nc


_CACHE: dict = {}


def _get_nc():
    if "nc" not in _CACHE:
        _CACHE["nc"] = build_module()
    return _CACHE["nc"]


def _in_maps(inputs):
    f32 = lambda a: np.ascontiguousarray(np.asarray(a), dtype=np.float32)
    x = f32(inputs["x"])
    shared = {
        "wq": f32(inputs["Wq"]), "bq": f32(inputs["bq"]),
        "wk": f32(inputs["Wk"]), "bk": f32(inputs["bk"]),
        "wv": f32(inputs["Wv"]), "bv": f32(inputs["bv"]),
        "wo": f32(inputs["Wo"]), "bo": f32(inputs["bo"]),
        "w1": f32(inputs["W1"]), "b1": f32(inputs["b1"]),
        "w2": f32(inputs["W2"]), "b2": f32(inputs["b2"]),
        "g1": f32(inputs["g1"]), "beta1": f32(inputs["beta1"]),
        "g2": f32(inputs["g2"]), "beta2": f32(inputs["beta2"]),
    }
    maps = []
    for c in range(NCORES):
        b, qi = divmod(c, NCORES // B)
        q0 = qi * SQ
        maps.append({
            "xb": np.ascontiguousarray(x[b]),
            "xq": np.ascontiguousarray(x[b, q0:q0 + SQ]),
            **shared,
        })
    return maps


def run(inputs, **kwargs):
    nc = _get_nc()
    res = run_bass_kernel_spmd(nc, _in_maps(inputs), core_ids=list(range(NCORES)), **kwargs)
    parts = [res.results[c]["out"] for c in range(NCORES)]
    y = np.concatenate(parts, axis=0).reshape(B, S, H).astype(np.float32)
    return y, res


def kernel(**inputs) -> np.ndarray:
    y, _ = run(inputs)
    return y


# revision 18
# speedup vs baseline: 1.2357x; 1.0853x over previous
"""Trainium2 Bass kernel for nn_EncodingLayer (dense transformer encoder layer).

Reference computation (B=2, S=2048, H=128, NH=8):
    Q/K/V = per-head full-dim projections of x, scores = QK^T/sqrt(H),
    A = softmax(scores), o = A@V, concat heads, y = o@Wo+bo,
    y = LN1(y), f = relu(relu(y@W1+b1)@W2+b2), out = LN2(y+f).

Sharding: data-parallel over query rows. Core c (of 8) owns batch b=c//4 and
query rows q0=(c%4)*512 .. q0+512 of that batch. Each core computes K/V for
its full batch (4x replicated compute, tiny) and the full epilogue for its
512 rows. No collectives; host concatenates the 8 [512,128] slices.

Within a core:
  Phase 0 (load): weights arrive as TWO host-packed [128, X] blobs (one DMA
    trigger costs ~950ns on the issuing engine, so 15 small DMAs would burn
    ~14us of issue time); x in two chunks. Triggers split across the two
    HWDGE engines (sync + scalar) so issue time overlaps.
  Phase 1 (projections, dense PE stream): x transposes, then V (natural
    [t, h, e], bf16) interleaved with K^T/Q^T per head (bf16). PSUM drains
    split ACT/DVE so neither throttles the PE stream.
  Phase 2 (attention): per head a pure t-loop: scores^T chunks on PE (bf16),
    P^T = exp on ACT straight out of PSUM (bf16), denominator via ones-vector
    matmul (sum over the t partition dim), o^T accumulated on PE. Softmax
    without max-subtraction is numerically exact at this problem scale
    (|scores|<~.4); bv folds into o^T after division because softmax rows sum
    to one. The denominator is staged PSUM->SBUF by ACT at head end so the
    next head's start=True matmul doesn't wait out the DVE reciprocal (WAR
    on the single denominator PSUM bank).
  Phase 3 (epilogue, fully transposed, two pipelined column halves): LN1/FFN/
    LN2 all in [feature, seq] layout; LN stats over the feature dim (= the
    partition dim) via ones-matmul column sums, so no transposes are needed
    until the final output. Elementwise LN tail split DVE/GpSimd.
"""

import math
import numpy as np
from contextlib import ExitStack

import concourse.bass as bass
import concourse.bacc as bacc
import concourse.mybir as mybir
import concourse.tile as tile
from concourse.bass_utils import run_bass_kernel_spmd
from concourse.masks import make_identity

B, S, H, NH = 2, 2048, 128, 8
F = 2 * H                      # FFN hidden dim (256)
NCORES = 8
SQ = (B * S) // NCORES         # 512 query rows per core
TC = S // 128                  # 16 key/value chunks of 128
CH = SQ // 2                   # epilogue column half
LN_EPS = 1e-5
FP32 = mybir.dt.float32
FP32R = mybir.dt.float32r
BF16 = mybir.dt.bfloat16
AF = mybir.ActivationFunctionType
ALU = mybir.AluOpType

# wall_a column offsets (fp32 cols): wv | wk | wq | bq | bk | bv
WA_WV, WA_WK, WA_WQ = 0, 1024, 2048
WA_BQ, WA_BK, WA_BV = 3072, 3080, 3088
WA_COLS = 3096
# wall_b: wo | w1 | w2 | bo | b1 | b2 | g1 | be1 | g2 | be2
WB_WO, WB_W1, WB_W2 = 0, 1024, 1280
WB_BO, WB_B1, WB_B2 = 1536, 1537, 1539
WB_G1, WB_BE1, WB_G2, WB_BE2 = 1540, 1541, 1542, 1543
WB_COLS = 1544


def _r(ap):
    return ap.bitcast(FP32R)


def build_module():
    nc = bacc.Bacc(None)

    xb_d = nc.declare_dram_parameter("xb", [S, H], FP32, isOutput=False)
    xq_d = nc.declare_dram_parameter("xq", [SQ, H], FP32, isOutput=False)
    wa_d = nc.declare_dram_parameter("wall_a", [128, WA_COLS], FP32R, isOutput=False)
    wb_d = nc.declare_dram_parameter("wall_b", [128, WB_COLS], FP32R, isOutput=False)
    out_d = nc.declare_dram_parameter("out", [SQ, H], FP32, isOutput=True)

    with tile.TileContext(nc) as tc, ExitStack() as ctx:
        singles = ctx.enter_context(tc.tile_pool(name="singles", bufs=1))

        # ---- constants ----
        ident = singles.tile([128, 128], FP32)
        make_identity(nc, ident[:])
        ones128 = singles.tile([128, 128], FP32)  # all-ones lhsT: partition sums
        nc.vector.memset(ones128[:], 1.0)
        ones_bf = singles.tile([128, 128], BF16)  # bf16 twin for bf16 matmuls
        nc.vector.memset(ones_bf[:], 1.0)
        eps_t = singles.tile([128, 1], FP32)
        nc.vector.memset(eps_t[:], LN_EPS)

        # ---- DMAs: x halves + packed weight blobs, split across engines ----
        xb_sb = singles.tile([128, TC, H], FP32)  # (s%128, sc, d)
        xb_r = xb_d[:].rearrange("(sc p) d -> p sc d", p=128)
        for q in range(2):
            nc.sync.dma_start(out=xb_sb[:, 8 * q:8 * (q + 1), :], in_=xb_r[:, 8 * q:8 * (q + 1), :])
        wa_sb = singles.tile([128, WA_COLS], FP32R)
        nc.sync.dma_start(out=wa_sb[:], in_=wa_d[:])
        xq_sb = singles.tile([128, SQ // 128, H], FP32)
        nc.scalar.dma_start(out=xq_sb[:], in_=xq_d[:].rearrange("(sc p) d -> p sc d", p=128))
        wb_sb = singles.tile([128, WB_COLS], FP32R)
        nc.scalar.dma_start(out=wb_sb[:], in_=wb_d[:])

        def wa(c0, n):          # fp32r view (matmul operands)
            return wa_sb[:, c0:c0 + n]

        def wb(c0, n):
            return wb_sb[:, c0:c0 + n]

        def waf(c0, n):         # plain-fp32 view (DVE/ACT/GpSimd operands)
            return wa_sb[:, c0:c0 + n].bitcast(FP32)

        def wbf(c0, n):
            return wb_sb[:, c0:c0 + n].bitcast(FP32)

        xT = singles.tile([H, S], FP32)
        xqT = singles.tile([H, SQ], FP32)

        # PE matmuls (fused LDWEIGHTS) can carry only ONE semaphore wait in
        # codegen. Dummy transposes/matmuls make PE observe one DMA/engine
        # semaphore so no later matmul needs to wait on two at once; _zd()
        # writes a [1,1] dummy into a new PSUM pool's first tile so the
        # pool-transition (released-zone) dependency is absorbed there
        # instead of landing on a real matmul that also has a data wait.
        def _zd(tile_ap):
            nc.tensor.matmul(tile_ap[0:1, 0:1], ident[:, 0:1], ident[:, 0:1],
                             start=True, stop=True)

        # ---- transposes: xT=[d, S], xqT=[d, SQ] ----
        with tc.tile_pool(name="tp_ps", bufs=2, space="PSUM") as tp_ps:
            pt0 = tp_ps.tile([128, 128], FP32, tag="abs")
            nc.tensor.transpose(pt0[:], ident[:], ident[:])          # observe ident
            nc.tensor.transpose(pt0[:], xb_sb[:, 0, :], ident[:])    # observe xb half 0
            for sc in range(TC):
                pt = tp_ps.tile([128, 128], FP32, tag="tp")
                if sc == 0:
                    _zd(pt)
                nc.tensor.transpose(pt[:], xb_sb[:, sc, :], ident[:])
                nc.vector.tensor_copy(out=_r(xT[:, sc * 128:(sc + 1) * 128]), in_=pt[:])
            for sc in range(SQ // 128):
                pt = tp_ps.tile([128, 128], FP32, tag="tp")
                nc.tensor.transpose(pt[:], xq_sb[:, sc, :], ident[:])
                nc.vector.tensor_copy(out=_r(xqT[:, sc * 128:(sc + 1) * 128]), in_=pt[:])
            # observe the wall_a DMA before the projection matmuls
            nc.tensor.transpose(pt0[:], wa_sb[:, 0:128].bitcast(FP32), ident[:])

        # ---- projection phase: V interleaved with K^T/Q^T ----
        v_sb = singles.tile([128, TC, NH, H], BF16)   # (t%128, tc, h, e)
        kt_all = singles.tile([H, NH, S], BF16)       # (e, h, t)
        qt_all = singles.tile([H, NH, SQ], BF16)      # (e, h, s)

        with (
            tc.tile_pool(name="v_ps", bufs=2, space="PSUM") as v_ps,
            tc.tile_pool(name="kq_ps", bufs=2, space="PSUM") as kq_ps,
        ):
            first = [True]

            def _v(tcc):
                vp = v_ps.tile([128, NH * H], FP32, tag="v")
                if first[0]:
                    _zd(vp)
                    first[0] = False
                for half in range(2):
                    nc.tensor.matmul(
                        vp[:, half * 512:(half + 1) * 512],
                        _r(xT[:, tcc * 128:(tcc + 1) * 128]),
                        wa(WA_WV + half * 512, 512),
                        start=True, stop=True,
                    )
                dst = v_sb[:, tcc, :, :]
                if tcc % 4 == 3:
                    nc.vector.tensor_copy(out=dst, in_=vp[:])
                else:
                    nc.scalar.activation(out=dst, in_=vp[:], func=AF.Copy)

            def _k(h, i):
                kp = kq_ps.tile([128, 1024], FP32, tag="kq")
                for j in range(2):
                    nc.tensor.matmul(
                        kp[:, j * 512:(j + 1) * 512],
                        wa(WA_WK + h * 128, 128),
                        _r(xT[:, (2 * i + j) * 512:(2 * i + j + 1) * 512]),
                        start=True, stop=True,
                    )
                nc.vector.tensor_scalar_add(
                    out=kt_all[:, h, i * 1024:(i + 1) * 1024], in0=kp[:],
                    scalar1=waf(WA_BK + h, 1),
                )

            def _q(h2):  # heads 2*h2, 2*h2+1 packed in one PSUM tile
                qp = kq_ps.tile([128, 1024], FP32, tag="kq")
                for hh in (2 * h2, 2 * h2 + 1):
                    nc.tensor.matmul(
                        qp[:, (hh % 2) * 512:(hh % 2 + 1) * 512],
                        wa(WA_WQ + hh * 128, 128), _r(xqT[:]),
                        start=True, stop=True,
                    )
                for hh in (2 * h2, 2 * h2 + 1):
                    nc.vector.tensor_scalar(
                        out=qt_all[:, hh, :], in0=qp[:, (hh % 2) * 512:(hh % 2 + 1) * 512],
                        scalar1=waf(WA_BQ + hh, 1), scalar2=1.0 / math.sqrt(H),
                        op0=ALU.add, op1=ALU.mult,
                    )

            # v0 k00 v1 k01 v2 k10 v3 k11 q01 | v4 ... q23 | ... | v15 k71 q67
            for blk in range(4):
                for i in range(4):
                    _v(4 * blk + i)
                    _k(2 * blk + i // 2, i % 2)
                _q(blk)

        # ---- attention head loop (pure t-loops) ----
        pt_pool = ctx.enter_context(tc.tile_pool(name="pt", bufs=3))
        ot_pool = ctx.enter_context(tc.tile_pool(name="ot", bufs=2))

        yT_sb = singles.tile([H, SQ], FP32)  # attention block output (pre-LN), [j, s]

        with (
            tc.tile_pool(name="s_ps", bufs=2, space="PSUM") as s_ps,
            tc.tile_pool(name="o_ps", bufs=2, space="PSUM") as o_ps,
            tc.tile_pool(name="d_ps", bufs=1, space="PSUM") as d_ps,
            tc.tile_pool(name="y_ps", bufs=1, space="PSUM") as y_ps,
        ):
            y_acc = y_ps.tile([H, SQ], FP32)
            _zd(y_acc)
            # Absorb wall_b and the DVE watermark of the last projection
            # drain (qt of head 7; DVE is in-order so this covers every
            # V/K/Q drain) once, before the head loops, so attention matmuls
            # carry at most one un-observed wait (ACT, for the pt tiles).
            gp_abs = s_ps.tile([128, 1024], FP32, tag="s")
            _zd(gp_abs)
            q_view = qt_all[0:1, NH - 1, 0:2].bitcast(FP32)[:, 0:1]
            nc.tensor.matmul(gp_abs[0:1, 0:1], q_view, q_view, start=True, stop=True)
            wb_view = wb_sb[0:1, 0:1].bitcast(FP32)
            nc.tensor.matmul(gp_abs[0:1, 1:2], wb_view, wb_view, start=True, stop=True)

            # o^T = o_acc / denom + bv (softmax rows sum to 1). Emitted at the
            # START of the next head so the DVE chain runs while PE streams the
            # next head's matmuls, and the Wo matmul is emitted AFTER that
            # head's t-loop so in-order PE never stalls on it.
            def _finalize_dve(hp, o_p, d_sb):
                rec_bc = ot_pool.tile([128, SQ], FP32, tag="rec")
                scr = ot_pool.tile([128, SQ], FP32, tag="rec_scr")
                nc.vector.reciprocal_approx_accurate(out=rec_bc[:], in_=d_sb[:], scratch=scr[:])
                oT = ot_pool.tile([H, SQ], FP32, tag="oT")
                nc.vector.tensor_mul(out=_r(oT[:]), in0=o_p[:], in1=rec_bc[:])
                nc.vector.tensor_scalar_add(out=_r(oT[:]), in0=oT[:],
                                            scalar1=waf(WA_BV + hp, 1))
                return oT

            prev = None  # (h, o_acc, d_sb)
            for h in range(NH):
                oT_prev = None
                kt = kt_all[:, h, :]
                qt = qt_all[:, h, :]

                o_acc = o_ps.tile([H, SQ], FP32, tag="o")
                d_acc = d_ps.tile([128, SQ], FP32, tag="d")
                if h == 0:
                    _zd(o_acc)
                    _zd(d_acc)

                # Software-pipelined t-loop: PE is in-order, so emit the
                # NEXT group's scores before this group's denominator/PV
                # matmuls — PE then has ~1.4us of work queued while ACT
                # computes exp(g), instead of stalling behind it.
                def _scores(g):
                    sp = s_ps.tile([128, 1024], FP32, tag="s")
                    for j in range(2):
                        tcc = 2 * g + j
                        nc.tensor.matmul(
                            sp[:, j * 512:(j + 1) * 512],
                            kt[:, tcc * 128:(tcc + 1) * 128], qt,
                            start=True, stop=True,
                        )
                    pt = pt_pool.tile([128, 1024], BF16, tag="pt")
                    nc.scalar.activation(out=pt[:], in_=sp[:], func=AF.Exp)
                    return pt

                def _denom_pv(g, pt):
                    for j in range(2):
                        tcc = 2 * g + j
                        nc.tensor.matmul(
                            d_acc[:], ones_bf[:], pt[:, j * 512:(j + 1) * 512],
                            start=(tcc == 0), stop=(tcc == TC - 1),
                        )
                    for j in range(2):
                        tcc = 2 * g + j
                        nc.tensor.matmul(
                            o_acc[:], v_sb[:, tcc, h, :], pt[:, j * 512:(j + 1) * 512],
                            start=(tcc == 0), stop=(tcc == TC - 1),
                        )

                pt_cur = _scores(0)
                for g in range(TC // 2):
                    if g == 2 and prev is not None:
                        oT_prev = _finalize_dve(*prev)
                    pt_next = _scores(g + 1) if g + 1 < TC // 2 else None
                    _denom_pv(g, pt_cur)
                    pt_cur = pt_next

                # Stage the finished denominator to SBUF on ACT (fast, and
                # frees the d_acc bank before the next head's start=True).
                d_sb = ot_pool.tile([128, SQ], FP32, tag="dsb")
                nc.scalar.activation(out=d_sb[:], in_=d_acc[:], func=AF.Copy)

                if prev is not None:
                    nc.tensor.matmul(y_acc[:], wb(WB_WO + prev[0] * 128, 128), _r(oT_prev[:]),
                                     start=(prev[0] == 0), stop=False)
                prev = (h, o_acc, d_sb)

            oT_last = _finalize_dve(*prev)
            nc.tensor.matmul(y_acc[:], wb(WB_WO + (NH - 1) * 128, 128), _r(oT_last[:]),
                             start=False, stop=True)
            for hf in range(2):
                sl = slice(hf * CH, (hf + 1) * CH)
                nc.vector.tensor_scalar_add(out=_r(yT_sb[:, sl]), in0=y_acc[:, sl],
                                            scalar1=wbf(WB_BO, 1))

        # ---- epilogue, fully transposed, two pipelined column halves ----
        epi = ctx.enter_context(tc.tile_pool(name="epi", bufs=1))

        def _ln_T(out_ap, in_ap, ps_pool, g_col, beta_col, hf, zd=False):
            t = str(hf)
            ysq = epi.tile([H, CH], FP32, tag="sq" + t)
            nc.scalar.activation(out=_r(ysq[:]), in_=in_ap, func=AF.Square)
            s_ps = ps_pool.tile([128, 2 * CH], FP32, tag="s" + t)
            if zd:
                _zd(s_ps)
            sum_y = s_ps[:, 0:CH]
            sum_q = s_ps[:, CH:2 * CH]
            nc.tensor.matmul(sum_y, _r(ones128[:]), _r(in_ap), start=True, stop=True)
            nc.tensor.matmul(sum_q, _r(ones128[:]), _r(ysq[:]), start=True, stop=True)
            # m = sum_y/128 (to SBUF: DVE reads at most one PSUM input/op),
            # t1 = m^2, var = sum_q/128 - m^2, std = sqrt(var + eps)
            m_sb = epi.tile([128, CH], FP32, tag="m" + t)
            nc.vector.tensor_scalar_mul(out=m_sb[:], in0=sum_y, scalar1=1.0 / H)
            t1 = epi.tile([128, CH], FP32, tag="t1" + t)
            nc.vector.scalar_tensor_tensor(
                out=t1[:], in0=sum_y, scalar=1.0 / H, in1=m_sb[:],
                op0=ALU.mult, op1=ALU.mult,
            )
            nc.vector.scalar_tensor_tensor(
                out=t1[:], in0=sum_q, scalar=1.0 / H, in1=t1[:],
                op0=ALU.mult, op1=ALU.subtract,
            )
            std = epi.tile([128, CH], FP32, tag="std" + t)
            nc.scalar.activation(out=std[:], in_=t1[:], func=AF.Sqrt, bias=eps_t[:])
            rstd = epi.tile([128, CH], FP32, tag="rs" + t)
            scr = epi.tile([128, CH], FP32, tag="scr" + t)
            nc.vector.reciprocal_approx_accurate(out=rstd[:], in_=std[:], scratch=scr[:])
            # elementwise tail on GpSimd (SBUF-only) to overlap with DVE
            ctr = epi.tile([128, CH], FP32, tag="ctr" + t)
            nc.gpsimd.tensor_sub(out=ctr[:], in0=in_ap, in1=m_sb[:])
            nc.gpsimd.tensor_mul(out=ctr[:], in0=ctr[:], in1=rstd[:])
            nc.gpsimd.tensor_scalar(
                out=out_ap, in0=ctr[:], scalar1=g_col, scalar2=beta_col,
                op0=ALU.mult, op1=ALU.add,
            )

        y1T = singles.tile([H, SQ], FP32)   # LN1 output, [j, s]
        uT = singles.tile([H, 2, SQ], FP32)
        rT = singles.tile([H, SQ], FP32)
        outT = singles.tile([H, SQ], FP32)
        out_sb = singles.tile([128, SQ // 128, H], FP32)
        out_r = out_d[:].rearrange("(sc p) j -> p sc j", p=128)

        with (
            tc.tile_pool(name="st_ps", bufs=1, space="PSUM") as st_ps,
            tc.tile_pool(name="u_ps", bufs=2, space="PSUM") as u_ps,
            tc.tile_pool(name="e_ps", bufs=2, space="PSUM") as e_ps,
        ):
            for hf in range(2):
                sl = slice(hf * CH, (hf + 1) * CH)
                _ln_T(_r(y1T[:, sl]), yT_sb[:, sl], st_ps,
                      wbf(WB_G1, 1), wbf(WB_BE1, 1), hf, zd=(hf == 0))

            for hf in range(2):
                sl = slice(hf * CH, (hf + 1) * CH)
                # u^T[f, s] = relu(W1^T y1 + b1), f in two 128-chunks
                up = u_ps.tile([128, 2 * CH], FP32, tag="u")
                if hf == 0:
                    _zd(up)
                for fc in range(2):
                    nc.tensor.matmul(up[:, fc * CH:(fc + 1) * CH], wb(WB_W1 + fc * 128, 128),
                                     _r(y1T[:, sl]), start=True, stop=True)
                    nc.scalar.activation(out=_r(uT[:, fc, sl]), in_=up[:, fc * CH:(fc + 1) * CH],
                                         func=AF.Relu, bias=wbf(WB_B1 + fc, 1))
                # z^T[j, s] = relu(W2^T u + b2)
                zp = u_ps.tile([H, CH], FP32, tag="z")
                for fc in range(2):
                    nc.tensor.matmul(zp[:], wb(WB_W2 + fc * 128, 128), _r(uT[:, fc, sl]),
                                     start=(fc == 0), stop=(fc == 1))
                # residual in transposed space: rT = y1T + relu(zp + b2)
                nc.scalar.activation(out=_r(rT[:, sl]), in_=zp[:], func=AF.Relu,
                                     bias=wbf(WB_B2, 1))
                nc.vector.tensor_add(out=_r(rT[:, sl]), in0=rT[:, sl], in1=y1T[:, sl])

            for hf in range(2):
                sl = slice(hf * CH, (hf + 1) * CH)
                _ln_T(outT[:, sl], rT[:, sl], st_ps,
                      wbf(WB_G2, 1), wbf(WB_BE2, 1), hf)

                # back to natural layout + store, per half
                for sc in range(hf * 2, hf * 2 + 2):
                    op = e_ps.tile([128, 128], FP32, tag="e")
                    if hf == 0 and sc == 0:
                        _zd(op)
                    nc.tensor.transpose(op[:], outT[:, sc * 128:(sc + 1) * 128], ident[:])
                    nc.vector.tensor_copy(out=out_sb[:, sc, :], in_=op[:])
                    nc.sync.dma_start(out=out_r[:, sc:sc + 1, :], in_=out_sb[:, sc:sc + 1, :])

    nc.finalize()
    return nc


_CACHE: dict = {}


def _get_nc():
    if "nc" not in _CACHE:
        _CACHE["nc"] = build_module()
    return _CACHE["nc"]


def _pack_walls(i):
    f32 = lambda a: np.ascontiguousarray(np.asarray(a), dtype=np.float32)
    wq, wk, wv = f32(i["Wq"]), f32(i["Wk"]), f32(i["Wv"])
    wall_a = np.concatenate([
        wv.transpose(1, 0, 2).reshape(H, NH * H),
        wk.transpose(1, 0, 2).reshape(H, NH * H),
        wq.transpose(1, 0, 2).reshape(H, NH * H),
        f32(i["bq"]).T, f32(i["bk"]).T, f32(i["bv"]).T,
    ], axis=1)
    wall_b = np.concatenate([
        f32(i["Wo"]).reshape(NH, H, H).transpose(1, 0, 2).reshape(H, NH * H),
        f32(i["W1"]),
        f32(i["W2"]).reshape(2, H, H).transpose(1, 0, 2).reshape(H, 2 * H),
        f32(i["bo"])[:, None],
        f32(i["b1"]).reshape(2, H).T,
        f32(i["b2"])[:, None],
        f32(i["g1"])[:, None], f32(i["beta1"])[:, None],
        f32(i["g2"])[:, None], f32(i["beta2"])[:, None],
    ], axis=1)
    assert wall_a.shape == (128, WA_COLS) and wall_b.shape == (128, WB_COLS)
    return np.ascontiguousarray(wall_a), np.ascontiguousarray(wall_b)


def _in_maps(inputs):
    x = np.ascontiguousarray(np.asarray(inputs["x"]), dtype=np.float32)
    wall_a, wall_b = _pack_walls(inputs)
    maps = []
    for c in range(NCORES):
        b, qi = divmod(c, NCORES // B)
        q0 = qi * SQ
        maps.append({
            "xb": np.ascontiguousarray(x[b]),
            "xq": np.ascontiguousarray(x[b, q0:q0 + SQ]),
            "wall_a": wall_a, "wall_b": wall_b,
        })
    return maps


def run(inputs, **kwargs):
    nc = _get_nc()
    res = run_bass_kernel_spmd(nc, _in_maps(inputs), core_ids=list(range(NCORES)), **kwargs)
    parts = [res.results[c]["out"] for c in range(NCORES)]
    y = np.concatenate(parts, axis=0).reshape(B, S, H).astype(np.float32)
    return y, res


def kernel(**inputs) -> np.ndarray:
    y, _ = run(inputs)
    return y


# revision 20
# speedup vs baseline: 1.3332x; 1.0789x over previous
"""Trainium2 Bass kernel for nn_EncodingLayer (dense transformer encoder layer).

Reference computation (B=2, S=2048, H=128, NH=8):
    Q/K/V = per-head full-dim projections of x, scores = QK^T/sqrt(H),
    A = softmax(scores), o = A@V, concat heads, y = o@Wo+bo,
    y = LN1(y), f = relu(relu(y@W1+b1)@W2+b2), out = LN2(y+f).

Sharding: data-parallel over query rows. Core c (of 8) owns batch b=c//4 and
query rows q0=(c%4)*512 .. q0+512 of that batch. Each core computes K for
its full batch (4x replicated compute, tiny) and the full epilogue for its
512 rows. No collectives; host concatenates the 8 [512,128] slices.

Key structure per core:
  Phase 0 (load): weights arrive as TWO host-packed [128, X] blobs (one DMA
    trigger costs ~950ns on the issuing engine, so 15 small DMAs would burn
    ~14us of issue time); x in two chunks. Triggers split across the two
    HWDGE engines (sync + scalar). There is also a fixed ~7us engine boot
    preamble before anything can run.
  Phase 1 (projections): x transposes, then K^T/Q^T per head (bf16), drains
    split ACT/DVE. V is NEVER projected: o = A(XWv) is reassociated as
    (AX)Wv, so the attention t-loop contracts P^T against natural-layout
    bf16 x chunks (same PE cost as A@V), and a single per-head [128,512]
    matmul applies Wv afterwards. This deletes the V projection matmuls and
    all 16 of their PSUM drains.
  Phase 2 (attention): per head a pure t-loop: scores^T chunks on PE (bf16),
    P^T = exp on ACT straight out of PSUM (bf16), denominator via ones-vector
    matmul (sum over the t partition dim), m = (AX)^T accumulated on PE.
    Softmax without max-subtraction is numerically exact at this problem
    scale (|scores|<~.4); bv folds in after division because softmax rows
    sum to one. The denominator is staged PSUM->SBUF by ACT at head end so
    the next head's start=True matmul doesn't wait out the DVE reciprocal.
  Phase 3 (epilogue, fully transposed, two pipelined column halves): LN1/FFN/
    LN2 all in [feature, seq] layout; LN stats over the feature dim (= the
    partition dim) via ones-matmul column sums, so no transposes are needed
    until the final output. The gain*x+beta step rides the ACT activation's
    per-partition scale/bias operands; second-half elementwise tail goes to
    GpSimd so the halves' chains overlap.
"""

import math
import numpy as np
from contextlib import ExitStack

import concourse.bass as bass
import concourse.bacc as bacc
import concourse.mybir as mybir
import concourse.tile as tile
from concourse.bass_utils import run_bass_kernel_spmd
from concourse.masks import make_identity

B, S, H, NH = 2, 2048, 128, 8
F = 2 * H                      # FFN hidden dim (256)
NCORES = 8
SQ = (B * S) // NCORES         # 512 query rows per core
TC = S // 128                  # 16 key chunks of 128
CH = SQ // 2                   # epilogue column half
LN_EPS = 1e-5
FP32 = mybir.dt.float32
FP32R = mybir.dt.float32r
BF16 = mybir.dt.bfloat16
AF = mybir.ActivationFunctionType
ALU = mybir.AluOpType

# wall_a column offsets (fp32 cols): wk | wq | wv | bq | bk | bv
WA_WK, WA_WQ, WA_WV = 0, 1024, 2048
WA_BQ, WA_BK, WA_BV = 3072, 3080, 3088
WA_COLS = 3096
# wall_b: wo | w1 | w2 | bo | b1 | b2 | g1 | be1 | g2 | be2
WB_WO, WB_W1, WB_W2 = 0, 1024, 1280
WB_BO, WB_B1, WB_B2 = 1536, 1537, 1539
WB_G1, WB_BE1, WB_G2, WB_BE2 = 1540, 1541, 1542, 1543
WB_COLS = 1544


def _r(ap):
    return ap.bitcast(FP32R)


def build_module():
    nc = bacc.Bacc(None)

    xb_d = nc.declare_dram_parameter("xb", [S, H], FP32, isOutput=False)
    xq_d = nc.declare_dram_parameter("xq", [SQ, H], FP32, isOutput=False)
    wa_d = nc.declare_dram_parameter("wall_a", [128, WA_COLS], FP32R, isOutput=False)
    wb_d = nc.declare_dram_parameter("wall_b", [128, WB_COLS], FP32R, isOutput=False)
    out_d = nc.declare_dram_parameter("out", [SQ, H], FP32, isOutput=True)

    with tile.TileContext(nc) as tc, ExitStack() as ctx:
        singles = ctx.enter_context(tc.tile_pool(name="singles", bufs=1))

        # ---- constants ----
        ident = singles.tile([128, 128], FP32)
        make_identity(nc, ident[:])
        ones128 = singles.tile([128, 128], FP32)  # all-ones lhsT: partition sums
        nc.vector.memset(ones128[:], 1.0)
        ones_bf = singles.tile([128, 128], BF16)  # bf16 twin for bf16 matmuls
        nc.vector.memset(ones_bf[:], 1.0)
        eps_t = singles.tile([128, 1], FP32)
        nc.vector.memset(eps_t[:], LN_EPS)

        # ---- DMAs: x halves + packed weight blobs, split across engines ----
        xb_sb = singles.tile([128, TC, H], FP32)  # (s%128, sc, d)
        xb_r = xb_d[:].rearrange("(sc p) d -> p sc d", p=128)
        for q in range(2):
            nc.sync.dma_start(out=xb_sb[:, 8 * q:8 * (q + 1), :], in_=xb_r[:, 8 * q:8 * (q + 1), :])
        wa_sb = singles.tile([128, WA_COLS], FP32R)
        nc.sync.dma_start(out=wa_sb[:], in_=wa_d[:])
        xq_sb = singles.tile([128, SQ // 128, H], FP32)
        nc.scalar.dma_start(out=xq_sb[:], in_=xq_d[:].rearrange("(sc p) d -> p sc d", p=128))
        wb_sb = singles.tile([128, WB_COLS], FP32R)
        nc.scalar.dma_start(out=wb_sb[:], in_=wb_d[:])

        def wa(c0, n):          # fp32r view (matmul operands)
            return wa_sb[:, c0:c0 + n]

        def wb(c0, n):
            return wb_sb[:, c0:c0 + n]

        def waf(c0, n):         # plain-fp32 view (DVE/ACT/GpSimd operands)
            return wa_sb[:, c0:c0 + n].bitcast(FP32)

        def wbf(c0, n):
            return wb_sb[:, c0:c0 + n].bitcast(FP32)

        xT = singles.tile([H, S], FP32)
        xqT = singles.tile([H, SQ], FP32)
        xb_bf = singles.tile([128, TC, H], BF16)  # natural x, bf16 (AX lhsT)

        # PE matmuls (fused LDWEIGHTS) can carry only ONE semaphore wait in
        # codegen. Dummy transposes/matmuls make PE observe one DMA/engine
        # semaphore so no later matmul needs to wait on two at once; _zd()
        # writes a [1,1] dummy into a new PSUM pool's first tile so the
        # pool-transition (released-zone) dependency is absorbed there
        # instead of landing on a real matmul that also has a data wait.
        def _zd(tile_ap):
            nc.tensor.matmul(tile_ap[0:1, 0:1], ident[:, 0:1], ident[:, 0:1],
                             start=True, stop=True)

        # ---- transposes: xT=[d, S], xqT=[d, SQ]; bf16 cast of natural x ----
        with tc.tile_pool(name="tp_ps", bufs=2, space="PSUM") as tp_ps:
            pt0 = tp_ps.tile([128, 128], FP32, tag="abs")
            nc.tensor.transpose(pt0[:], ident[:], ident[:])          # observe ident
            nc.tensor.transpose(pt0[:], xb_sb[:, 0, :], ident[:])    # observe xb half 0
            for sc in range(TC):
                pt = tp_ps.tile([128, 128], FP32, tag="tp")
                if sc == 0:
                    _zd(pt)
                nc.tensor.transpose(pt[:], xb_sb[:, sc, :], ident[:])
                nc.vector.tensor_copy(out=_r(xT[:, sc * 128:(sc + 1) * 128]), in_=pt[:])
                if sc % 4 == 3:
                    nc.vector.tensor_copy(out=xb_bf[:, sc - 3:sc + 1, :],
                                          in_=xb_sb[:, sc - 3:sc + 1, :])
            for sc in range(SQ // 128):
                pt = tp_ps.tile([128, 128], FP32, tag="tp")
                nc.tensor.transpose(pt[:], xq_sb[:, sc, :], ident[:])
                nc.vector.tensor_copy(out=_r(xqT[:, sc * 128:(sc + 1) * 128]), in_=pt[:])
            # observe the wall_a DMA before the projection matmuls
            nc.tensor.transpose(pt0[:], wa_sb[:, 0:128].bitcast(FP32), ident[:])

        # ---- projection phase: K^T and Q^T only ----
        kt_all = singles.tile([H, NH, S], BF16)       # (e, h, t)
        qt_all = singles.tile([H, NH, SQ], BF16)      # (e, h, s)

        with tc.tile_pool(name="kq_ps", bufs=4, space="PSUM") as kq_ps:
            first = [True]

            def _k(h, i):
                kp = kq_ps.tile([128, 1024], FP32, tag="kq")
                if first[0]:
                    _zd(kp)
                    first[0] = False
                for j in range(2):
                    nc.tensor.matmul(
                        kp[:, j * 512:(j + 1) * 512],
                        wa(WA_WK + h * 128, 128),
                        _r(xT[:, (2 * i + j) * 512:(2 * i + j + 1) * 512]),
                        start=True, stop=True,
                    )
                dst = kt_all[:, h, i * 1024:(i + 1) * 1024]
                if (2 * h + i) % 2 == 0:
                    nc.vector.tensor_scalar_add(out=dst, in0=kp[:], scalar1=waf(WA_BK + h, 1))
                else:
                    nc.scalar.activation(out=dst, in_=kp[:], func=AF.Identity,
                                         bias=waf(WA_BK + h, 1))

            def _q(h2):  # heads 2*h2, 2*h2+1 packed in one PSUM tile
                qp = kq_ps.tile([128, 1024], FP32, tag="kq")
                for hh in (2 * h2, 2 * h2 + 1):
                    nc.tensor.matmul(
                        qp[:, (hh % 2) * 512:(hh % 2 + 1) * 512],
                        wa(WA_WQ + hh * 128, 128), _r(xqT[:]),
                        start=True, stop=True,
                    )
                for hh in (2 * h2, 2 * h2 + 1):
                    nc.vector.tensor_scalar(
                        out=qt_all[:, hh, :], in0=qp[:, (hh % 2) * 512:(hh % 2 + 1) * 512],
                        scalar1=waf(WA_BQ + hh, 1), scalar2=1.0 / math.sqrt(H),
                        op0=ALU.add, op1=ALU.mult,
                    )

            for blk in range(4):
                for hh in range(2):
                    h = 2 * blk + hh
                    _k(h, 0)
                    _k(h, 1)
                _q(blk)

        # ---- attention head loop (pure t-loops) ----
        pt_pool = ctx.enter_context(tc.tile_pool(name="pt", bufs=3))
        ot_pool = ctx.enter_context(tc.tile_pool(name="ot", bufs=2))

        yT_sb = singles.tile([H, SQ], FP32)  # attention block output (pre-LN), [j, s]

        with (
            tc.tile_pool(name="s_ps", bufs=2, space="PSUM") as s_ps,
            tc.tile_pool(name="m_ps", bufs=1, space="PSUM") as m_ps,
            tc.tile_pool(name="op_ps", bufs=1, space="PSUM") as op_ps,
            tc.tile_pool(name="d_ps", bufs=1, space="PSUM") as d_ps,
            tc.tile_pool(name="y_ps", bufs=1, space="PSUM") as y_ps,
        ):
            y_acc = y_ps.tile([H, SQ], FP32)
            _zd(y_acc)
            # Absorb wall_b and the DVE watermark of the last projection
            # drain (qt of head 7; DVE is in-order so this covers every
            # K/Q drain and the xb_bf cast) once, before the head loops, so
            # attention matmuls carry at most one un-observed wait.
            gp_abs = s_ps.tile([128, 1024], FP32, tag="s")
            _zd(gp_abs)
            q_view = qt_all[0:1, NH - 1, 0:2].bitcast(FP32)[:, 0:1]
            nc.tensor.matmul(gp_abs[0:1, 0:1], q_view, q_view, start=True, stop=True)
            wb_view = wb_sb[0:1, 0:1].bitcast(FP32)
            nc.tensor.matmul(gp_abs[0:1, 1:2], wb_view, wb_view, start=True, stop=True)

            # o^T = op / denom + bv (softmax rows sum to 1). Emitted at the
            # START of the next head so the DVE chain runs while PE streams
            # the next head's matmuls; the Wo matmul is emitted at the END of
            # that head's t-loop so in-order PE never stalls on it.
            def _finalize_dve(hp, op_p, d_sb):
                rec_bc = ot_pool.tile([128, SQ], FP32, tag="rec")
                scr = ot_pool.tile([128, SQ], FP32, tag="rec_scr")
                nc.vector.reciprocal_approx_accurate(out=rec_bc[:], in_=d_sb[:], scratch=scr[:])
                oT = ot_pool.tile([H, SQ], FP32, tag="oT")
                nc.vector.tensor_mul(out=_r(oT[:]), in0=op_p[:], in1=rec_bc[:])
                nc.vector.tensor_scalar_add(out=_r(oT[:]), in0=oT[:],
                                            scalar1=waf(WA_BV + hp, 1))
                return oT

            prev = None  # (h, op_tile, d_sb)
            for h in range(NH):
                oT_prev = None
                kt = kt_all[:, h, :]
                qt = qt_all[:, h, :]

                m_acc = m_ps.tile([H, SQ], FP32, tag="m")
                d_acc = d_ps.tile([128, SQ], FP32, tag="d")
                if h == 0:
                    _zd(m_acc)
                    _zd(d_acc)

                # Software-pipelined t-loop: PE is in-order, so emit the
                # NEXT group's scores before this group's denominator/AX
                # matmuls — PE then has ~1.4us of work queued while ACT
                # computes exp(g), instead of stalling behind it.
                def _scores(g):
                    sp = s_ps.tile([128, 1024], FP32, tag="s")
                    for j in range(2):
                        tcc = 2 * g + j
                        nc.tensor.matmul(
                            sp[:, j * 512:(j + 1) * 512],
                            kt[:, tcc * 128:(tcc + 1) * 128], qt,
                            start=True, stop=True,
                        )
                    pt = pt_pool.tile([128, 1024], BF16, tag="pt")
                    nc.scalar.activation(out=pt[:], in_=sp[:], func=AF.Exp)
                    return pt

                def _denom_ax(g, pt):
                    for j in range(2):
                        tcc = 2 * g + j
                        nc.tensor.matmul(
                            d_acc[:], ones_bf[:], pt[:, j * 512:(j + 1) * 512],
                            start=(tcc == 0), stop=(tcc == TC - 1),
                        )
                    for j in range(2):
                        tcc = 2 * g + j
                        nc.tensor.matmul(
                            m_acc[:], xb_bf[:, tcc, :], pt[:, j * 512:(j + 1) * 512],
                            start=(tcc == 0), stop=(tcc == TC - 1),
                        )

                pt_cur = _scores(0)
                for g in range(TC // 2):
                    if g == 2 and prev is not None:
                        oT_prev = _finalize_dve(*prev)
                    pt_next = _scores(g + 1) if g + 1 < TC // 2 else None
                    _denom_ax(g, pt_cur)
                    pt_cur = pt_next

                # m = (AX)^T to SBUF (DVE), then one matmul applies Wv.
                m_sb = ot_pool.tile([H, SQ], FP32, tag="msb")
                nc.vector.tensor_copy(out=_r(m_sb[:]), in_=m_acc[:])
                # Stage the finished denominator to SBUF on ACT (fast, and
                # frees the d_acc bank before the next head's start=True).
                d_sb = ot_pool.tile([128, SQ], FP32, tag="dsb")
                nc.scalar.activation(out=d_sb[:], in_=d_acc[:], func=AF.Copy)

                if prev is not None:
                    nc.tensor.matmul(y_acc[:], wb(WB_WO + prev[0] * 128, 128), _r(oT_prev[:]),
                                     start=(prev[0] == 0), stop=False)
                op_tile = op_ps.tile([H, SQ], FP32, tag="op")
                nc.tensor.matmul(op_tile[:], wa(WA_WV + h * 128, 128), _r(m_sb[:]),
                                 start=True, stop=True)
                prev = (h, op_tile, d_sb)

            oT_last = _finalize_dve(*prev)
            nc.tensor.matmul(y_acc[:], wb(WB_WO + (NH - 1) * 128, 128), _r(oT_last[:]),
                             start=False, stop=True)
            for hf in range(2):
                sl = slice(hf * CH, (hf + 1) * CH)
                nc.vector.tensor_scalar_add(out=_r(yT_sb[:, sl]), in0=y_acc[:, sl],
                                            scalar1=wbf(WB_BO, 1))

        # ---- epilogue, fully transposed, two pipelined column halves ----
        epi = ctx.enter_context(tc.tile_pool(name="epi", bufs=1))

        def _ln_stats(in_ap, ps_pool, hf, zd=False):
            t = str(hf)
            ysq = epi.tile([H, CH], FP32, tag="sq" + t)
            nc.scalar.activation(out=_r(ysq[:]), in_=in_ap, func=AF.Square)
            s_ps = ps_pool.tile([128, 2 * CH], FP32, tag="s" + t)
            if zd:
                _zd(s_ps)
            sum_y = s_ps[:, 0:CH]
            sum_q = s_ps[:, CH:2 * CH]
            nc.tensor.matmul(sum_y, _r(ones128[:]), _r(in_ap), start=True, stop=True)
            nc.tensor.matmul(sum_q, _r(ones128[:]), _r(ysq[:]), start=True, stop=True)
            m_sb = epi.tile([128, CH], FP32, tag="m" + t)
            nc.vector.tensor_scalar_mul(out=m_sb[:], in0=sum_y, scalar1=1.0 / H)
            t1 = epi.tile([128, CH], FP32, tag="t1" + t)
            nc.vector.scalar_tensor_tensor(
                out=t1[:], in0=sum_y, scalar=1.0 / H, in1=m_sb[:],
                op0=ALU.mult, op1=ALU.mult,
            )
            nc.vector.scalar_tensor_tensor(
                out=t1[:], in0=sum_q, scalar=1.0 / H, in1=t1[:],
                op0=ALU.mult, op1=ALU.subtract,
            )
            std = epi.tile([128, CH], FP32, tag="std" + t)
            nc.scalar.activation(out=std[:], in_=t1[:], func=AF.Sqrt, bias=eps_t[:])
            rstd = epi.tile([128, CH], FP32, tag="rs" + t)
            scr = epi.tile([128, CH], FP32, tag="scr" + t)
            nc.vector.reciprocal_approx_accurate(out=rstd[:], in_=std[:], scratch=scr[:])
            return m_sb, rstd

        def _ln_tail(out_ap, in_ap, m_sb, rstd, g_col, beta_col, hf):
            # (x-m)*rstd on DVE (half 0) / GpSimd (half 1); gain+shift rides
            # ACT's per-partition scale/bias.
            t = str(hf)
            eng = nc.vector if hf == 0 else nc.gpsimd
            ctr = epi.tile([128, CH], FP32, tag="ctr" + t)
            eng.tensor_sub(out=ctr[:], in0=in_ap, in1=m_sb[:])
            eng.tensor_mul(out=ctr[:], in0=ctr[:], in1=rstd[:])
            nc.scalar.activation(out=out_ap, in_=ctr[:], func=AF.Identity,
                                 scale=g_col, bias=beta_col)

        y1T = singles.tile([H, SQ], FP32)   # LN1 output, [j, s]
        uT = singles.tile([H, 2, SQ], FP32)
        rT = singles.tile([H, SQ], FP32)
        outT = singles.tile([H, SQ], FP32)
        out_sb = singles.tile([128, SQ // 128, H], FP32)
        out_r = out_d[:].rearrange("(sc p) j -> p sc j", p=128)

        with (
            tc.tile_pool(name="st_ps", bufs=1, space="PSUM") as st_ps,
            tc.tile_pool(name="u_ps", bufs=2, space="PSUM") as u_ps,
            tc.tile_pool(name="e_ps", bufs=2, space="PSUM") as e_ps,
        ):
            mr = [None, None]
            for hf in range(2):
                sl = slice(hf * CH, (hf + 1) * CH)
                mr[hf] = _ln_stats(yT_sb[:, sl], st_ps, hf, zd=(hf == 0))
            for hf in range(2):
                sl = slice(hf * CH, (hf + 1) * CH)
                _ln_tail(_r(y1T[:, sl]), yT_sb[:, sl], *mr[hf],
                         wbf(WB_G1, 1), wbf(WB_BE1, 1), hf)

            for hf in range(2):
                sl = slice(hf * CH, (hf + 1) * CH)
                # u^T[f, s] = relu(W1^T y1 + b1), f in two 128-chunks
                up = u_ps.tile([128, 2 * CH], FP32, tag="u")
                if hf == 0:
                    _zd(up)
                for fc in range(2):
                    nc.tensor.matmul(up[:, fc * CH:(fc + 1) * CH], wb(WB_W1 + fc * 128, 128),
                                     _r(y1T[:, sl]), start=True, stop=True)
                    nc.scalar.activation(out=_r(uT[:, fc, sl]), in_=up[:, fc * CH:(fc + 1) * CH],
                                         func=AF.Relu, bias=wbf(WB_B1 + fc, 1))
                # z^T[j, s] = relu(W2^T u + b2)
                zp = u_ps.tile([H, CH], FP32, tag="z")
                for fc in range(2):
                    nc.tensor.matmul(zp[:], wb(WB_W2 + fc * 128, 128), _r(uT[:, fc, sl]),
                                     start=(fc == 0), stop=(fc == 1))
                # residual in transposed space: rT = y1T + relu(zp + b2)
                nc.scalar.activation(out=_r(rT[:, sl]), in_=zp[:], func=AF.Relu,
                                     bias=wbf(WB_B2, 1))
                nc.vector.tensor_add(out=_r(rT[:, sl]), in0=rT[:, sl], in1=y1T[:, sl])

            for hf in range(2):
                sl = slice(hf * CH, (hf + 1) * CH)
                mr[hf] = _ln_stats(rT[:, sl], st_ps, hf)
            for hf in range(2):
                sl = slice(hf * CH, (hf + 1) * CH)
                _ln_tail(outT[:, sl], rT[:, sl], *mr[hf],
                         wbf(WB_G2, 1), wbf(WB_BE2, 1), hf)

                # back to natural layout + store, per half
                for sc in range(hf * 2, hf * 2 + 2):
                    op = e_ps.tile([128, 128], FP32, tag="e")
                    if hf == 0 and sc == 0:
                        _zd(op)
                    nc.tensor.transpose(op[:], outT[:, sc * 128:(sc + 1) * 128], ident[:])
                    nc.vector.tensor_copy(out=out_sb[:, sc, :], in_=op[:])
                    nc.sync.dma_start(out=out_r[:, sc:sc + 1, :], in_=out_sb[:, sc:sc + 1, :])

    nc.finalize()
    return nc


_CACHE: dict = {}


def _get_nc():
    if "nc" not in _CACHE:
        _CACHE["nc"] = build_module()
    return _CACHE["nc"]


def _pack_walls(i):
    f32 = lambda a: np.ascontiguousarray(np.asarray(a), dtype=np.float32)
    wall_a = np.concatenate([
        f32(i["Wk"]).transpose(1, 0, 2).reshape(H, NH * H),
        f32(i["Wq"]).transpose(1, 0, 2).reshape(H, NH * H),
        f32(i["Wv"]).transpose(1, 0, 2).reshape(H, NH * H),
        f32(i["bq"]).T, f32(i["bk"]).T, f32(i["bv"]).T,
    ], axis=1)
    wall_b = np.concatenate([
        f32(i["Wo"]).reshape(NH, H, H).transpose(1, 0, 2).reshape(H, NH * H),
        f32(i["W1"]),
        f32(i["W2"]).reshape(2, H, H).transpose(1, 0, 2).reshape(H, 2 * H),
        f32(i["bo"])[:, None],
        f32(i["b1"]).reshape(2, H).T,
        f32(i["b2"])[:, None],
        f32(i["g1"])[:, None], f32(i["beta1"])[:, None],
        f32(i["g2"])[:, None], f32(i["beta2"])[:, None],
    ], axis=1)
    assert wall_a.shape == (128, WA_COLS) and wall_b.shape == (128, WB_COLS)
    return np.ascontiguousarray(wall_a), np.ascontiguousarray(wall_b)


def _in_maps(inputs):
    x = np.ascontiguousarray(np.asarray(inputs["x"]), dtype=np.float32)
    wall_a, wall_b = _pack_walls(inputs)
    maps = []
    for c in range(NCORES):
        b, qi = divmod(c, NCORES // B)
        q0 = qi * SQ
        maps.append({
            "xb": np.ascontiguousarray(x[b]),
            "xq": np.ascontiguousarray(x[b, q0:q0 + SQ]),
            "wall_a": wall_a, "wall_b": wall_b,
        })
    return maps


def run(inputs, **kwargs):
    nc = _get_nc()
    res = run_bass_kernel_spmd(nc, _in_maps(inputs), core_ids=list(range(NCORES)), **kwargs)
    parts = [res.results[c]["out"] for c in range(NCORES)]
    y = np.concatenate(parts, axis=0).reshape(B, S, H).astype(np.float32)
    return y, res


def kernel(**inputs) -> np.ndarray:
    y, _ = run(inputs)
    return y
